# revision 4
# baseline (speedup 1.0000x reference)
"""GQA attention kernel for Trainium2, 8 NeuronCores.

Sharding: batch (2) x q-head-groups (4) = 8 cores. Each core handles one
batch and 8 q-heads (2 kv-heads). Host passes x^T so every matmul flows
without on-chip transposes. Scores are computed transposed [k, q]; softmax
denominator comes from a fused [V|ones] matmul; causal masking via 4
precomputed additive mask tiles. bv/bo folded in host-side (exact since
softmax rows sum to one).
"""
import sys
import numpy as np

if "/opt/trn_rl_repo" not in sys.path:
    sys.path.insert(0, "/opt/trn_rl_repo")

S = 2048
HID = 2048
D = 64
PERM = [0, 4, 1, 5, 2, 6, 3, 7]  # position -> local head (parity matches kv head)
NEG = -1.0e30


def _build_program():
    import concourse.tile as tile
    from concourse import bacc, mybir

    F32 = mybir.dt.float32
    AFT = mybir.ActivationFunctionType

    nc = bacc.Bacc(None, target_bir_lowering=False, debug=False)
    xT = nc.declare_dram_parameter("xT", [HID, S], F32, isOutput=False)
    wqT = nc.declare_dram_parameter("wqT", [HID, 512], F32, isOutput=False)
    wkT = nc.declare_dram_parameter("wkT", [HID, 128], F32, isOutput=False)
    wvT = nc.declare_dram_parameter("wvT", [HID, 128], F32, isOutput=False)
    woT = nc.declare_dram_parameter("woT", [512, HID], F32, isOutput=False)
    bqp = nc.declare_dram_parameter("bqp", [128, 4], F32, isOutput=False)
    bkp = nc.declare_dram_parameter("bkp", [128, 1], F32, isOutput=False)
    msk = nc.declare_dram_parameter("msk", [128, 2048], F32, isOutput=False)
    outp = nc.declare_dram_parameter("outp", [S, HID], F32, isOutput=True)

    with tile.TileContext(nc) as tc:
        with tc.tile_pool(name="res", bufs=1) as res:
            qt_sb = res.tile([128, 4 * 2048], F32)   # QT row-tile m -> cols m*2048..
            kt_sb = res.tile([128, 2048], F32)       # KT: 128 kv dims x tokens
            v_sb = res.tile([128, 16 * 130], F32)    # per token tile: [V0|1|V1|1]
            at_sb = res.tile([128, 4 * 2048], F32)   # attnT, same layout as qt_sb
            mk_sb = res.tile([128, 2048], F32)       # 4 causal mask tiles
            bq_sb = res.tile([128, 4], F32)
            bk_sb = res.tile([128, 1], F32)
            one_sb = res.tile([1, 64], F32)

            nc.sync.dma_start(out=mk_sb, in_=msk[:, :])
            nc.sync.dma_start(out=bq_sb, in_=bqp[:, :])
            nc.sync.dma_start(out=bk_sb, in_=bkp[:, :])
            nc.vector.memset(one_sb, 1.0)
            for t in range(16):
                nc.vector.memset(v_sb[:, t * 130 + 64: t * 130 + 65], 1.0)
                nc.vector.memset(v_sb[:, t * 130 + 129: t * 130 + 130], 1.0)

            # ---------------- Phase 1: Q/K/V projections ----------------
            with tc.tile_pool(name="xt", bufs=8) as xtp, \
                 tc.tile_pool(name="wq", bufs=8) as wqp, \
                 tc.tile_pool(name="wkv", bufs=16) as wkvp, \
                 tc.tile_pool(name="qps", bufs=4, space="PSUM") as qps, \
                 tc.tile_pool(name="kps", bufs=2, space="PSUM") as kps, \
                 tc.tile_pool(name="vps", bufs=2, space="PSUM") as vps:
                for kkb in range(2):
                    xts, wqs, wks, wvs = [], [], [], []
                    for j in range(8):
                        kk = kkb * 8 + j
                        xt_t = xtp.tile([128, 2048], F32, tag="xt")
                        nc.sync.dma_start(out=xt_t, in_=xT[kk * 128:(kk + 1) * 128, :])
                        xts.append(xt_t)
                        wq_t = wqp.tile([128, 512], F32, tag="wq")
                        nc.sync.dma_start(out=wq_t, in_=wqT[kk * 128:(kk + 1) * 128, :])
                        wqs.append(wq_t)
                        wk_t = wkvp.tile([128, 128], F32, tag="wk")
                        nc.sync.dma_start(out=wk_t, in_=wkT[kk * 128:(kk + 1) * 128, :])
                        wks.append(wk_t)
                        wv_t = wkvp.tile([128, 128], F32, tag="wv")
                        nc.sync.dma_start(out=wv_t, in_=wvT[kk * 128:(kk + 1) * 128, :])
                        wvs.append(wv_t)
                    for n in range(4):
                        tok = slice(n * 512, (n + 1) * 512)
                        for m in range(4):
                            q_t = qps.tile([128, 512], F32, tag="qps")
                            for j in range(8):
                                nc.tensor.matmul(q_t, wqs[j][:, m * 128:(m + 1) * 128],
                                                 xts[j][:, tok], start=(j == 0), stop=(j == 7))
                            dst = qt_sb[:, m * 2048 + n * 512: m * 2048 + (n + 1) * 512]
                            if kkb == 0:
                                nc.scalar.activation(dst, q_t, AFT.Identity, bias=bq_sb[:, m:m + 1])
                            else:
                                nc.vector.tensor_add(dst, dst, q_t)
                        k_t = kps.tile([128, 512], F32, tag="kps")
                        for j in range(8):
                            nc.tensor.matmul(k_t, wks[j], xts[j][:, tok], start=(j == 0), stop=(j == 7))
                        dstk = kt_sb[:, n * 512:(n + 1) * 512]
                        if kkb == 0:
                            nc.scalar.activation(dstk, k_t, AFT.Identity, bias=bk_sb[:, 0:1])
                        else:
                            nc.vector.tensor_add(dstk, dstk, k_t)
                        v_t = vps.tile([128, 512], F32, tag="vps")
                        for st in range(4):
                            for j in range(8):
                                nc.tensor.matmul(v_t[:, st * 128:(st + 1) * 128],
                                                 xts[j][:, n * 512 + st * 128: n * 512 + (st + 1) * 128],
                                                 wvs[j], start=(j == 0), stop=(j == 7))
                        for st in range(4):
                            t = n * 4 + st
                            for g in range(2):
                                dstv = v_sb[:, t * 130 + g * 65: t * 130 + g * 65 + 64]
                                src = v_t[:, st * 128 + g * 64: st * 128 + g * 64 + 64]
                                if kkb == 0:
                                    nc.vector.tensor_copy(dstv, src)
                                else:
                                    nc.vector.tensor_add(dstv, dstv, src)

            # ---------------- Phase 2: attention per head ----------------
            with tc.tile_pool(name="ex", bufs=3) as exp, \
                 tc.tile_pool(name="sm", bufs=4) as smp, \
                 tc.tile_pool(name="scps", bufs=3, space="PSUM") as scps, \
                 tc.tile_pool(name="atps", bufs=2, space="PSUM") as atps, \
                 tc.tile_pool(name="dbps", bufs=2, space="PSUM") as dbps:
                for pos in range(8):
                    kvh = pos % 2
                    mq, rq = pos // 2, (pos % 2) * 64
                    for jq in range(4):
                        nk = 4 * jq + 4
                        at_t = atps.tile([65, 512], F32, tag="at")
                        q_rhs = qt_sb[rq:rq + 64, mq * 2048 + jq * 512: mq * 2048 + (jq + 1) * 512]
                        for i in range(nk):
                            sc_t = scps.tile([128, 512], F32, tag="sc")
                            nc.tensor.matmul(sc_t, kt_sb[kvh * 64:kvh * 64 + 64, i * 128:(i + 1) * 128],
                                             q_rhs, start=True, stop=True)
                            ex_t = exp.tile([128, 512], F32, tag="ex")
                            r = i - 4 * jq
                            if r >= 0:
                                tmp = exp.tile([128, 512], F32, tag="tmp")
                                nc.vector.tensor_add(tmp, sc_t, mk_sb[:, r * 512:(r + 1) * 512])
                                nc.scalar.activation(ex_t, tmp, AFT.Exp, scale=0.125)
                            else:
                                nc.scalar.activation(ex_t, sc_t, AFT.Exp, scale=0.125)
                            nc.tensor.matmul(at_t, v_sb[:, i * 130 + kvh * 65: i * 130 + kvh * 65 + 65],
                                             ex_t, start=(i == 0), stop=(i == nk - 1))
                        # normalize: den = at_t row 64 -> base 0 via sbuf-sbuf DMA
                        cp_t = smp.tile([65, 512], F32, tag="cp")
                        nc.vector.tensor_copy(cp_t[64:65, :], at_t[64:65, :])
                        dn_t = smp.tile([1, 512], F32, tag="dn")
                        nc.sync.dma_start(out=dn_t, in_=cp_t[64:65, :])
                        dr_t = smp.tile([1, 512], F32, tag="dr")
                        nc.vector.reciprocal(dr_t, dn_t)
                        db_t = dbps.tile([64, 512], F32, tag="db")
                        nc.tensor.matmul(db_t, one_sb, dr_t, start=True, stop=True)
                        db_sb = smp.tile([64, 512], F32, tag="dbs")
                        nc.vector.tensor_copy(db_sb, db_t)
                        blk = slice(mq * 2048 + jq * 512, mq * 2048 + (jq + 1) * 512)
                        if rq == 0:
                            nc.vector.tensor_mul(at_sb[0:64, blk], at_t[0:64, :], db_sb)
                        else:
                            sh_t = smp.tile([64, 512], F32, tag="sh")
                            nc.vector.tensor_mul(sh_t, at_t[0:64, :], db_sb)
                            nc.sync.dma_start(out=at_sb[64:128, blk], in_=sh_t)

            # ---------------- Phase 3: output projection ----------------
            with tc.tile_pool(name="wo", bufs=4) as wop, \
                 tc.tile_pool(name="ou", bufs=2) as oup, \
                 tc.tile_pool(name="ops", bufs=4, space="PSUM") as ops:
                wos = []
                for kk in range(4):
                    wo_t = wop.tile([128, 2048], F32, tag="wo")
                    nc.sync.dma_start(out=wo_t, in_=woT[kk * 128:(kk + 1) * 128, :])
                    wos.append(wo_t)
                for mt in range(16):
                    o_t = oup.tile([128, 2048], F32, tag="ou")
                    for nn in range(4):
                        op_t = ops.tile([128, 512], F32, tag="op")
                        for kk in range(4):
                            nc.tensor.matmul(op_t, at_sb[:, kk * 2048 + mt * 128: kk * 2048 + (mt + 1) * 128],
                                             wos[kk][:, nn * 512:(nn + 1) * 512],
                                             start=(kk == 0), stop=(kk == 3))
                        nc.vector.tensor_copy(o_t[:, nn * 512:(nn + 1) * 512], op_t)
                    nc.sync.dma_start(out=outp[mt * 128:(mt + 1) * 128, :], in_=o_t)

    nc.compile()
    return nc


def _mask_tiles():
    kp = np.arange(128)[:, None]
    qf = np.arange(512)[None, :]
    cols = []
    for r in range(4):
        cols.append(np.where(kp <= qf - r * 128, 0.0, NEG).astype(np.float32))
    return np.concatenate(cols, axis=1)  # [128, 2048]


def _prepare_in_maps(inputs):
    x = np.asarray(inputs["x"], dtype=np.float32)
    Wq = np.asarray(inputs["Wq"], dtype=np.float32)
    Wk = np.asarray(inputs["Wk"], dtype=np.float32)
    Wv = np.asarray(inputs["Wv"], dtype=np.float32)
    Wo = np.asarray(inputs["Wo"], dtype=np.float32)
    bq = np.asarray(inputs["bq"], dtype=np.float32)
    bk = np.asarray(inputs["bk"], dtype=np.float32)
    msk = _mask_tiles()

    in_maps = []
    for c in range(8):
        b, qg = c // 4, c % 4
        gh = [qg * 8 + PERM[p] for p in range(8)]
        qrows = np.concatenate([np.arange(h * 64, (h + 1) * 64) for h in gh])
        kvrows = np.arange(qg * 128, (qg + 1) * 128)
        in_maps.append({
            "xT": np.ascontiguousarray(x[b].T),
            "wqT": np.ascontiguousarray(Wq[qrows, :].T),
            "wkT": np.ascontiguousarray(Wk[kvrows, :].T),
            "wvT": np.ascontiguousarray(Wv[kvrows, :].T),
            "woT": np.ascontiguousarray(Wo[:, qrows].T),
            "bqp": np.ascontiguousarray(bq[qrows].reshape(4, 128).T),
            "bkp": np.ascontiguousarray(bk[kvrows].reshape(128, 1)),
            "msk": msk,
        })
    return in_maps


def kernel(**inputs):
    from concourse.bass_utils import run_bass_kernel_spmd

    Wo = np.asarray(inputs["Wo"], dtype=np.float32)
    bv = np.asarray(inputs["bv"], dtype=np.float32)
    bo = np.asarray(inputs["bo"], dtype=np.float32)

    nc = _build_program()
    in_maps = _prepare_in_maps(inputs)

    res = run_bass_kernel_spmd(nc, in_maps, list(range(8)))
    global _LAST_EXEC_NS, _LAST_RESULTS
    _LAST_EXEC_NS = res.exec_time_ns
    _LAST_RESULTS = res

    out = np.zeros((2, S, HID), np.float32)
    for c, r in enumerate(res.results):
        out[c // 4] += r["outp"]
    bv_full = np.repeat(bv.reshape(8, D), 4, axis=0).reshape(-1)
    out += (bv_full @ Wo.T + bo)[None, None, :]
    return out
